# revision 2
# baseline (speedup 1.0000x reference)
"""Bahdanau attention on 8 Trainium2 NeuronCores (bf16 single-pass).

Data-parallel over batch: each core handles B_L = B/8 = 4 batches, weights
replicated.  One fused pass over enc (flash-style; no max subtraction —
scores are O(+-5) so exp() is safe in f32):

Per batch b, per s-chunk (512 rows):
  enc_bf [128s,4,2048e] bf16 <- gpsimd cast-DMA (f32 HBM read, bf16 SBUF
                                write; the ONLY read of enc)
  encT   [128e,16,512s] bf16 <- PE transpose + DVE drains
  keyT   [128h,512s]    f32   = sum_e WkT[e,h] encT[e,s]    (PE bf16)
  T      = tanh(keyT + qT[h,b])                             (ACT bias)
  sps    [1,512]        = sum_h WeT[h] T[h,s]               (PE f32r M=1)
  esc    = exp(sps), accum_out -> chunk sum                 (ACT)
  expT   [128s,4] bf16 <- DRAM roundtrip of esc (sync DMAs + DVE cast)
  cps    [1,4,512]     += sum_s expT[s] * enc_bf[s,e]       (PE bf16, PSUM
                          accumulated across all 16 s-blocks of the batch)
Finalize: out = cps * (1/sum exp)  (ACT scale; sync DMA out)

ctx-matmul emission lags two chunks so the exp roundtrip hides under the
next chunks' transposes.  enc loads ride the gpsimd (SWDGE) queue; the small
roundtrip/output DMAs ride sync (HWDGE) — neither blocks the other.
"""

import sys

if "/opt/trn_rl_repo" not in sys.path:
    sys.path.insert(0, "/opt/trn_rl_repo")

import os

import numpy as np

import concourse.bass as bass
import concourse.mybir as mybir
from concourse import bacc
from concourse.tile import TileContext
from concourse.bass_utils import run_bass_kernel_spmd
from concourse.masks import make_identity

B, S, H = 32, 2048, 1024
E = 2 * H
N_CORES = 8
B_L = B // N_CORES       # 4 batches per core
SC = 512                 # s-chunk
NSC = S // SC            # 4
SB = S // 128            # 16 s-blocks of 128
EC = E // 128            # 16 e-blocks of 128
HC = H // 128            # 8
EQ = E // 512            # 4 e-quarters for ctx

F32 = mybir.dt.float32
F32R = mybir.dt.float32r
BF16 = mybir.dt.bfloat16
ACT_F = mybir.ActivationFunctionType
AX = mybir.AxisListType

_CACHE = {}


def _build(repeat=1):
    key = ("nc", repeat)
    if key in _CACHE:
        return _CACHE[key]
    nc = bacc.Bacc("TRN2", target_bir_lowering=False, debug=False,
                   num_devices=N_CORES)
    enc = nc.dram_tensor("enc", [B_L, S, E], F32, kind="ExternalInput").ap()
    dec = nc.dram_tensor("dec", [B_L, H], F32, kind="ExternalInput").ap()
    wq = nc.dram_tensor("Wq", [H, H], F32, kind="ExternalInput").ap()
    wk = nc.dram_tensor("Wk", [H, E], F32, kind="ExternalInput").ap()
    we = nc.dram_tensor("We", [1, H], F32, kind="ExternalInput").ap()
    out = nc.dram_tensor("out", [B_L, E], F32, kind="ExternalOutput").ap()

    with TileContext(nc) as tc:
        with (
            tc.tile_pool(name="const", bufs=1) as cpool,
            tc.tile_pool(name="dram", bufs=2, space="DRAM") as dpool,
            tc.tile_pool(name="encbf", bufs=4) as epool,
            tc.tile_pool(name="encT", bufs=2) as etpool,
            tc.tile_pool(name="tpool", bufs=1) as tpool,
            tc.tile_pool(name="scores", bufs=2) as scpool,
            tc.tile_pool(name="small", bufs=4) as small,
            tc.tile_pool(name="tr_psum", bufs=2, space="PSUM") as tr_psum,
            tc.tile_pool(name="key_psum", bufs=2, space="PSUM") as key_psum,
            tc.tile_pool(name="ctx_psum", bufs=1, space="PSUM") as ctx_psum,
        ):
            # ---------------- setup (outside repeat loop) ----------------
            ident = cpool.tile([128, 128], F32)
            make_identity(nc, ident)
            identb = cpool.tile([128, 128], BF16)
            nc.vector.tensor_copy(identb[:], ident[:])

            # WkT [128e, EC, H] bf16, resident.  Load wk rows with a casting
            # DMA, transpose on PE.
            wkT = cpool.tile([128, EC, H], BF16)
            for hg in range(2):
                for eh in range(2):
                    wbf = epool.tile([128, 4, E], BF16, name="encbf")
                    for j in range(4):
                        hc = hg * 4 + j
                        nc.gpsimd.dma_start(
                            wbf[:, j, :H],
                            wk[hc * 128:(hc + 1) * 128, eh * H:(eh + 1) * H])
                    for ec8 in range(8):
                        ec = eh * 8 + ec8
                        psb = tr_psum.tile([128, 512], BF16, name="trb")
                        for j in range(4):
                            nc.tensor.transpose(
                                psb[:, j * 128:(j + 1) * 128],
                                wbf[:, j, ec8 * 128:(ec8 + 1) * 128],
                                identb[:])
                        nc.vector.tensor_copy(
                            wkT[:, ec, hg * 512:(hg + 1) * 512], psb[:])

            # qT[h, b] = sum_e WqT[e,h] dec[b,e]   ([128, HC, B_L] f32 ACT
            # bias), computed in f32/f32r for accuracy.
            decT = cpool.tile([128, HC, B_L], F32R)
            for b2 in range(B_L):
                nc.sync.dma_start(
                    decT[:, :, b2],
                    dec.bitcast(F32R)[b2, :].rearrange("(c p) -> p c", p=128))
            qT = cpool.tile([128, HC, B_L], F32)
            wqst = cpool.tile([128, H], F32)
            for hc in range(HC):
                nc.sync.dma_start(wqst[:],
                                  wq[hc * 128:(hc + 1) * 128, :])
                qps = key_psum.tile([128, SC], F32, name="key")
                for ec in range(HC):
                    ps = tr_psum.tile([128, 512], F32, name="trb")
                    nc.tensor.transpose(ps[:, :128],
                                        wqst[:, ec * 128:(ec + 1) * 128],
                                        ident[:])
                    blk = small.tile([128, 128], F32R, name="blk")
                    nc.vector.tensor_copy(blk[:, :], ps[:, :128])
                    nc.tensor.matmul(qps[:, :B_L], blk[:, :], decT[:, ec, :],
                                     start=(ec == 0), stop=(ec == HC - 1))
                nc.vector.tensor_copy(qT[:, hc, :], qps[:, :B_L])

            # WeT : [128, HC] f32r
            weT = cpool.tile([128, HC], F32R)
            nc.sync.dma_start(
                weT[:], we.bitcast(F32R).rearrange("o (c p) -> p (o c)", p=128))

            # persistent tanh tiles
            T_sb = [tpool.tile([128, SC], F32R, name=f"T{hc}") for hc in range(HC)]

            # ---------------- per-iteration body ----------------
            def emit_load_tr(b, sc):
                """Cast-DMA an s-chunk of enc to bf16, transpose to encT."""
                ebf = epool.tile([128, 4, E], BF16, name="encbf")
                for ss in range(4):
                    r0 = sc * SC + ss * 128
                    nc.gpsimd.dma_start(ebf[:, ss, :], enc[b, r0:r0 + 128, :])
                encT = etpool.tile([128, EC, SC], BF16, name="encT")
                for ec in range(EC):
                    psb = tr_psum.tile([128, 512], BF16, name="trb")
                    for ss in range(4):
                        nc.tensor.transpose(psb[:, ss * 128:(ss + 1) * 128],
                                            ebf[:, ss, ec * 128:(ec + 1) * 128],
                                            identb[:])
                    nc.vector.tensor_copy(encT[:, ec, :], psb[:])
                return ebf, encT

            def emit_key_scores(b, sc, encT, zparts):
                """bf16 key matmul, tanh, scores, exp, expT roundtrip."""
                for hc in range(HC):
                    kps = key_psum.tile([128, SC], F32, name="key")
                    for ec in range(EC):
                        nc.tensor.matmul(
                            kps[:], wkT[:, ec, hc * 128:(hc + 1) * 128],
                            encT[:, ec, :],
                            start=(ec == 0), stop=(ec == EC - 1))
                    nc.scalar.activation(T_sb[hc][:], kps[:], ACT_F.Tanh,
                                         bias=qT[:, hc, b:b + 1])
                sps = key_psum.tile([128, SC], F32, name="key")[0:1, :]
                for hc in range(HC):
                    nc.tensor.matmul(sps[:], weT[:, hc:hc + 1], T_sb[hc][:],
                                     start=(hc == 0), stop=(hc == HC - 1))
                esc = scpool.tile([1, SC], F32, name="esc")
                nc.scalar.activation(esc[:], sps[:], ACT_F.Exp,
                                     accum_out=zparts[:, sc:sc + 1])
                adram = dpool.tile([SC], F32, name="adram")
                nc.sync.dma_start(adram[:], esc[:])
                expT = small.tile([128, 4], F32R, name="expT")
                nc.sync.dma_start(
                    expT[:], adram.bitcast(F32R).rearrange("(c p) -> p c", p=128))
                expTb = small.tile([128, 4], BF16, name="expTb")
                nc.vector.tensor_copy(expTb[:], expT[:])
                return expTb

            def emit_ctx(b, sc, ebf, expTb, cps):
                for ss in range(4):
                    for eq in range(EQ):
                        nc.tensor.matmul(
                            cps[:, eq, :], expTb[:, ss:ss + 1],
                            ebf[:, ss, eq * 512:(eq + 1) * 512],
                            start=(sc == 0 and ss == 0),
                            stop=(sc == NSC - 1 and ss == 3))

            def emit_finalize(b, cps, zparts):
                zsum = small.tile([1, 1], F32, name="zsum")
                nc.vector.reduce_sum(zsum[:], zparts[:], axis=AX.X)
                rinv = small.tile([1, 1], F32, name="rinv")
                nc.vector.reciprocal(rinv[:], zsum[:])
                osb = scpool.tile([1, E], F32, name="osb")
                nc.scalar.activation(osb[:], cps[:, :, :], ACT_F.Copy,
                                     scale=rinv[:])
                nc.sync.dma_start(out[b:b + 1, :], osb[:])

            def body():
                # ctx emission lags TWO chunks so the exp roundtrip has a
                # full chunk of slack before its ctx matmuls run.
                pend = []            # [(b, sc, ebf, expTb, cps, zparts), ...]
                LAG = 2

                def drain_one():
                    b0, sc0, ebf0, expTb0, cps0, zp0 = pend.pop(0)
                    emit_ctx(b0, sc0, ebf0, expTb0, cps0)
                    if sc0 == NSC - 1:
                        emit_finalize(b0, cps0, zp0)

                for b in range(B_L):
                    cps = ctx_psum.tile([1, EQ, 512], F32, name="ctx")
                    zparts = small.tile([1, NSC], F32, name="zparts")
                    for sc in range(NSC):
                        ebf, encT = emit_load_tr(b, sc)
                        while len(pend) >= LAG:
                            drain_one()
                        expTb = emit_key_scores(b, sc, encT, zparts)
                        pend.append((b, sc, ebf, expTb, cps, zparts))
                while pend:
                    drain_one()

            if repeat == 1:
                body()
            else:
                with tc.For_i(0, repeat, 1):
                    body()

    nc.compile()
    _CACHE[key] = nc
    return nc


def _shard_inputs(encoder_output, decoder_hidden, Wq, Wk, We):
    enc = np.ascontiguousarray(encoder_output, dtype=np.float32)
    dec = np.ascontiguousarray(decoder_hidden, dtype=np.float32).reshape(B, H)
    Wq = np.ascontiguousarray(Wq, dtype=np.float32)
    Wk = np.ascontiguousarray(Wk, dtype=np.float32)
    We = np.ascontiguousarray(We, dtype=np.float32)
    in_maps = []
    for c in range(N_CORES):
        sl = slice(c * B_L, (c + 1) * B_L)
        in_maps.append({
            "enc": enc[sl], "dec": dec[sl], "Wq": Wq, "Wk": Wk, "We": We,
        })
    return in_maps


def kernel(encoder_output, decoder_hidden, Wq, Wk, We):
    nc = _build()
    in_maps = _shard_inputs(encoder_output, decoder_hidden, Wq, Wk, We)
    res = run_bass_kernel_spmd(nc, in_maps, core_ids=list(range(N_CORES)))
    outs = [res.results[c]["out"] for c in range(N_CORES)]
    full = np.concatenate(outs, axis=0).reshape(B, 1, E).astype(np.float32)
    return full
